# revision 14
# baseline (speedup 1.0000x reference)
"""AdaptiveGaussianConvLayer Trainium2 kernel (8 NeuronCores, SPMD, no collectives).

Math: out[b, j, d] = sum_i V[b, i, d] * W[b, i, j],
      W[b, i, j] = exp(-0.5 * ((j - i - mu[b,i]) / sigma[b,i])^2)
with B=4, N=4096, D=512; sigma in (0.5, 2.5), mu ~ 3*N(0,1).

W underflows to exactly 0.0 in fp32 once |j - i - mu|/sigma >= ~13.2, i.e. for
|j - i| >= ~48.  On a 64-shifted slab grid (slab s = rows [128s - 64, 128s +
64) of the core's j-range), each 128-wide j-tile t needs only slabs {t, t+1},
so the banded result matches the dense reference to fp32 rounding.

Sharding: 8 cores = (batch b) x (j-half h).  Core c computes
out[b, h*2048:(h+1)*2048, :].  Host pads V/sigma/mu with 64 zero rows on each
side of the core's i-window so all cores run one identical SPMD program.

Single-pass W on ACT: Derivative_Erf(x) = (2/sqrt(pi)) * exp(-x^2), so with
x = z/sqrt(2):  W = (sqrt(pi)/2) * Derivative_Erf(z / sqrt(2)).  ACT computes
f(scale*u + bias) with per-partition scale/bias, so one activation per slab
(scale r' = 1/(sigma*sqrt(2)), bias b0' = (-64 - p - mu) * r') produces the
slab's W directly in bf16 — no Square pass, no Exp pass, no z2 buffers.  The
sqrt(pi)/2 correction is folded into V on the host (V is pre-cast to bf16
there anyway, halving its DMA traffic).

Output is written in bf16 (the matmul already runs in bf16; measured rel err
~5e-4 vs the 2e-2 gate), halving out-DMA bytes; the host upcasts to fp32.

Per-core dataflow (i on partitions, j/d on the free axis):
  W slab s = DErf(r'_s * iota + b0'_s)        (ACT, bf16 out, 17 instrs)
  psum t   = sum_{k=0,1} W[slab t+k].T @ V[slab t+k]   (TensorE, K=128 bf16)
  obuf     <- psum bf16 copy (DVE evens / GpSimd odds), DMA out in 2-tile
              pairs on the sync ring (V's queue, so V keeps priority)
A few scratch matmuls warm the PE clock gate before the real stream begins.
"""

import os
import numpy as np
import ml_dtypes

import concourse.bass as bass
import concourse.bacc as bacc
import concourse.mybir as mybir
import concourse.tile as tile
from concourse.bass_utils import run_bass_kernel_spmd

AF = mybir.ActivationFunctionType
ALU = mybir.AluOpType

B, N, D = 4, 4096, 512
NCORES = 8
HALF = N // 2             # 2048 j per core
NSLAB = HALF // 128 + 1   # 17 slabs of 128 rows on the 64-shifted grid
VROWS = NSLAB * 128       # 2176
JT = HALF // 128          # 16 j-tiles per core
WWIN = 256                # j-window width per slab
CW = 2 * NSLAB            # cst columns: (b0', r') pairs (iota is on-chip)

SQRT2 = float(np.sqrt(2.0))
WSCALE = float(np.sqrt(np.pi) / 2.0)

# genuinely used j-window per slab (edge slabs serve one j-tile)
def _slab_win(s):
    t_lo, t_hi = max(s - 1, 0), min(s, JT - 1)
    lo = (t_lo - (s - 1)) * 128
    return lo, (t_hi - t_lo + 1) * 128

WARMUP = int(os.environ.get("AGC_WARMUP", "5"))
FLATBAR = os.environ.get("AGC_FLATBAR", "1") == "1"
# PSUM->SBUF copy engine per tile: v=DVE (inline), a=ACT (deferred until
# after the last W slab so the W stream never stalls).  Only DVE/ACT have
# PSUM read ports (Pool TensorCopy from PSUM fails BIR verification).
COPYMAP = os.environ.get("AGC_COPYMAP", "v" * 13 + "a" * 3)

_cached = {}


def _flat_start_barrier(self, *, sem_only=False):
    """Flat all-engine barrier: every engine incs one sem and waits for the
    full count — one cross-engine hop instead of the stock sequential chain."""
    arrive = self.alloc_semaphore("flat_barrier_arrive")
    n = len(self.engines)
    for eng in self.engines.values():
        eng.sem_inc(arrive, 1)
    for eng in self.engines.values():
        eng.wait_ge(arrive, n)
    if not hasattr(self, "_flat_barrier_sems"):
        self._flat_barrier_sems = []
    self._flat_barrier_sems.append(arrive)


_stock_drain_and_barrier = tile.TileContext._drain_and_barrier


def _tail_drain_and_barrier(self, tick_clock, wait_clock):
    """Stock tail (its barrier instructions order the in-flight DMA completion
    sems ahead of the clears) + clear the flat-start-barrier sem so
    re-execution starts from zero."""
    _stock_drain_and_barrier(self, tick_clock, wait_clock)
    nc = self.nc
    fs = getattr(nc, "_flat_barrier_sems", [])
    if fs:
        nc.clear_and_free_semaphores(fs)
        nc._flat_barrier_sems = []


def build_nc():
    tile.TileContext._drain_and_barrier = _tail_drain_and_barrier
    f32 = mybir.dt.float32
    bf16 = mybir.dt.bfloat16
    orig_barrier = bass.Bass.all_engine_barrier
    if FLATBAR:
        bass.Bass.all_engine_barrier = _flat_start_barrier
    try:
        nc = bacc.Bacc("TRN2", target_bir_lowering=False, debug=False)
    finally:
        bass.Bass.all_engine_barrier = orig_barrier

    # V pre-scaled by sqrt(pi)/2, pre-cast to bf16 AND pre-tiled partition-
    # major on the host: Vp[p, s*D+d] = V[row 128s+p, d] — every partition is
    # one contiguous run per DMA slice (big descriptors -> full DMA bandwidth)
    vp_d = nc.dram_tensor("Vp", [128, NSLAB * D], bf16, kind="ExternalInput").ap()
    # cst = [iota(256) | b0' r' pairs] per partition
    cst_d = nc.dram_tensor("cst", [128, CW], f32, kind="ExternalInput").ap()
    out_d = nc.dram_tensor("out", [HALF, D], bf16, kind="ExternalOutput").ap()

    with tile.TileContext(nc) as tc:
        with (
            tc.tile_pool(name="const", bufs=1) as constp,
            tc.tile_pool(name="big", bufs=1) as bigp,
            tc.tile_pool(name="ps", bufs=8, space=bass.MemorySpace.PSUM) as pspool,
            tc.tile_pool(name="obuf", bufs=8) as opool,
        ):
            cst_t = constp.tile([128, CW], f32, name="cst_t")
            b0r = lambda s: (cst_t[:, 2 * s : 2 * s + 1],
                             cst_t[:, 2 * s + 1 : 2 * s + 2])

            vbuf = bigp.tile([128, NSLAB * D], bf16, name="vbuf")
            wbuf = bigp.tile([128, NSLAB * WWIN], bf16, name="wbuf")

            # cst (tiny b0'/r' table) first on the SYNC ring — measured
            # issue->bytes latency there is ~0.3us vs ~2.5us on the ACT and
            # GpSimd rings, and being first keeps the W chain off the
            # descriptor-generation queue behind V's 2.2MB.  The
            # auto-inserted erf_derivative ACT_TABLE_LOAD runs while the
            # cst bytes are in flight.
            nc.sync.dma_start(cst_t[:], cst_d[:])

            # iota source row for the W activations, generated on-chip
            # (fp32 is exact for 0..255)
            iota_t = constp.tile([128, WWIN], f32, name="iota_t")
            nc.gpsimd.iota(iota_t[:], [[1, WWIN]], base=0, channel_multiplier=0,
                           allow_small_or_imprecise_dtypes=True)

            # PE warm-up: scratch matmuls on zeros so the clock gate has
            # ramped when the real matmul stream begins.  Memsets on DVE —
            # it is otherwise idle until its first PSUM copy.
            wscr = bigp.tile([128, 128], bf16, name="wscr")
            nc.vector.memset(wscr[:], 0.0)
            wscr2 = bigp.tile([128, D], bf16, name="wscr2")
            nc.vector.memset(wscr2[:], 0.0)
            wps = pspool.tile([128, D], f32, tag="ps", name="wps")
            for _ in range(WARMUP):
                nc.tensor.matmul(wps[:, 0:D], wscr[:], wscr2[:],
                                 start=True, stop=True)

            # V loads all on the sync ring behind cst, slab order =
            # consumption order.  The out pairs ride the same queue behind
            # V, so V keeps priority.
            for lo, hi in ((0, 4), (4, 8), (8, 12), (12, 17)):
                nc.sync.dma_start(vbuf[:, lo * D : hi * D], vp_d[:, lo * D : hi * D])

            # W slab s in one ACT pass: DErf(r'*u + b0') = (2/sqrt(pi)) *
            # exp(-((u - 64 - p - mu)/sigma)^2 / 2)
            def emit_w(s):
                lo, w = _slab_win(s)
                b0, r = b0r(s)
                nc.scalar.activation(
                    wbuf[:, s * WWIN + lo : s * WWIN + lo + w],
                    iota_t[:, lo : lo + w],
                    AF.Derivative_Erf, bias=b0, scale=r)

            out3 = out_d.rearrange("(P h p) d -> P p h d", h=2, p=128)

            def emit_jtile(t, ps):
                out = ps[:]
                for k in range(2):
                    ls = t + k
                    nc.tensor.matmul(
                        out,
                        wbuf[:, ls * WWIN + (1 - k) * 128 : ls * WWIN + (2 - k) * 128],
                        vbuf[:, ls * D : (ls + 1) * D],
                        start=(k == 0),
                        stop=(k == 1),
                    )

            def emit_copy(t, ps, ob):
                dst = ob[:, (t % 2) * D : (t % 2 + 1) * D]
                if COPYMAP[t] == "a":
                    nc.scalar.activation(dst, ps[:], AF.Copy)
                else:
                    nc.vector.tensor_copy(dst, ps[:])

            def emit_out_dma(t, ob):
                if t == 14:
                    nc.sync.dma_start(out3[7, :, 0, :], ob[:, 0:D])
                elif t == 15:
                    nc.scalar.dma_start(out3[7, :, 1, :], ob[:, D : 2 * D])
                elif t % 2 == 1:
                    nc.sync.dma_start(
                        out3[t // 2],
                        ob[:].rearrange("p (h d) -> p h d", h=2))

            # pipeline: per-slab W -> j-tiles as they unlock -> copy -> DMA.
            # Out pairs ride the sync ring behind V (V keeps priority).
            # ACT-owned copies (and their DMAs) are deferred until after the
            # last W slab so the W stream never stalls PE.
            emit_w(0)
            ps = ob = None
            deferred = []
            for s in range(1, NSLAB):
                emit_w(s)
                t = s - 1
                ps = pspool.tile([128, D], f32, tag="ps", name="ps")
                if t % 2 == 0:
                    ob = opool.tile([128, 2 * D], bf16, name="ob")
                emit_jtile(t, ps)
                if COPYMAP[t] == "a":
                    deferred.append((t, ps, ob))
                else:
                    emit_copy(t, ps, ob)
                    emit_out_dma(t, ob)
            for t, ps, ob in deferred:
                emit_copy(t, ps, ob)
                emit_out_dma(t, ob)

    nc.compile()
    return nc


def _get_nc():
    if "nc" not in _cached:
        _cached["nc"] = build_nc()
    return _cached["nc"]


def make_in_maps(V, sigma, mu):
    """Host-side sharding: per-core padded bf16 V rows + scale table."""
    V = np.asarray(V, dtype=np.float32)
    sigma = np.asarray(sigma, dtype=np.float32).reshape(B, N)
    mu = np.asarray(mu, dtype=np.float32).reshape(B, N)
    pidx = (np.arange(VROWS) % 128).astype(np.float32)
    in_maps = []
    for c in range(NCORES):
        b, h = divmod(c, 2)
        jb = h * HALF
        lo, hi = jb - 64, jb + HALF + 64
        slo, shi = max(lo, 0), min(hi, N)
        vp = np.zeros((VROWS, D), ml_dtypes.bfloat16)
        sig = np.ones(VROWS, np.float32)
        muv = np.zeros(VROWS, np.float32)
        vp[slo - lo : shi - lo] = (V[b, slo:shi] * WSCALE).astype(ml_dtypes.bfloat16)
        sig[slo - lo : shi - lo] = sigma[b, slo:shi]
        muv[slo - lo : shi - lo] = mu[b, slo:shi]
        r = (np.float32(1.0) / (sig * np.float32(SQRT2))).astype(np.float32)
        b0 = ((np.float32(-64.0) - pidx - muv) * r).astype(np.float32)
        cst = np.zeros((128, CW), np.float32)
        cst[:, 0 : 2 * NSLAB : 2] = b0.reshape(NSLAB, 128).T
        cst[:, 1 : 2 * NSLAB : 2] = r.reshape(NSLAB, 128).T
        vp2 = np.ascontiguousarray(
            vp.reshape(NSLAB, 128, D).transpose(1, 0, 2).reshape(128, NSLAB * D))
        in_maps.append({"Vp": vp2, "cst": cst})
    return in_maps


def gather(results):
    out = np.empty((B, N, D), np.float32)
    for c in range(NCORES):
        b, h = divmod(c, 2)
        out[b, h * HALF : (h + 1) * HALF] = np.asarray(
            results[c]["out"]).astype(np.float32)
    return out


def kernel(V, sigma, mu):
    nc = _get_nc()
    in_maps = make_in_maps(V, sigma, mu)
    res = run_bass_kernel_spmd(nc, in_maps, core_ids=list(range(NCORES)))
    return gather(res.results)


# revision 15
# speedup vs baseline: 1.0414x; 1.0414x over previous
"""AdaptiveGaussianConvLayer Trainium2 kernel (8 NeuronCores, SPMD, no collectives).

Math: out[b, j, d] = sum_i V[b, i, d] * W[b, i, j],
      W[b, i, j] = exp(-0.5 * ((j - i - mu[b,i]) / sigma[b,i])^2)
with B=4, N=4096, D=512; sigma in (0.5, 2.5), mu ~ 3*N(0,1).

W underflows to exactly 0.0 in fp32 once |j - i - mu|/sigma >= ~13.2, i.e. for
|j - i| >= ~48.  On a 64-shifted slab grid (slab s = rows [128s - 64, 128s +
64) of the core's j-range), each 128-wide j-tile t needs only slabs {t, t+1},
so the banded result matches the dense reference to fp32 rounding.

Sharding: 8 cores = (batch b) x (j-half h).  Core c computes
out[b, h*2048:(h+1)*2048, :].  Host pads V/sigma/mu with 64 zero rows on each
side of the core's i-window so all cores run one identical SPMD program.

Single-pass W on ACT: Derivative_Erf(x) = (2/sqrt(pi)) * exp(-x^2), so with
x = z/sqrt(2):  W = (sqrt(pi)/2) * Derivative_Erf(z / sqrt(2)).  ACT computes
f(scale*u + bias) with per-partition scale/bias, so one activation per slab
(scale r' = 1/(sigma*sqrt(2)), bias b0' = (-64 - p - mu) * r') produces the
slab's W directly in bf16 — no Square pass, no Exp pass, no z2 buffers.  The
sqrt(pi)/2 correction is folded into V on the host (V is pre-cast to bf16
there anyway, halving its DMA traffic).

Output is written in bf16 (the matmul already runs in bf16; measured rel err
~5e-4 vs the 2e-2 gate), halving out-DMA bytes; the host upcasts to fp32.

Per-core dataflow (i on partitions, j/d on the free axis):
  W slab s = DErf(r'_s * iota + b0'_s)        (ACT, bf16 out, 17 instrs)
  psum t   = sum_{k=0,1} W[slab t+k].T @ V[slab t+k]   (TensorE, K=128 bf16)
  obuf     <- psum bf16 copy (DVE evens / GpSimd odds), DMA out in 2-tile
              pairs on the sync ring (V's queue, so V keeps priority)
A few scratch matmuls warm the PE clock gate before the real stream begins.
"""

import os
import numpy as np
import ml_dtypes

import concourse.bass as bass
import concourse.bacc as bacc
import concourse.mybir as mybir
import concourse.tile as tile
from concourse.bass_utils import run_bass_kernel_spmd

AF = mybir.ActivationFunctionType
ALU = mybir.AluOpType

B, N, D = 4, 4096, 512
NCORES = 8
HALF = N // 2             # 2048 j per core
NSLAB = HALF // 128 + 1   # 17 slabs of 128 rows on the 64-shifted grid
VROWS = NSLAB * 128       # 2176
JT = HALF // 128          # 16 j-tiles per core
WWIN = 256                # j-window width per slab
CW = 2 * NSLAB            # cst columns: (b0', r') pairs (iota is on-chip)

SQRT2 = float(np.sqrt(2.0))
WSCALE = float(np.sqrt(np.pi) / 2.0)

# genuinely used j-window per slab (edge slabs serve one j-tile)
def _slab_win(s):
    t_lo, t_hi = max(s - 1, 0), min(s, JT - 1)
    lo = (t_lo - (s - 1)) * 128
    return lo, (t_hi - t_lo + 1) * 128

WARMUP = int(os.environ.get("AGC_WARMUP", "5"))
FLATBAR = os.environ.get("AGC_FLATBAR", "1") == "1"
# PSUM->SBUF copy engine per tile: v=DVE (inline), a=ACT (deferred until
# after the last W slab so the W stream never stalls).  Only DVE/ACT have
# PSUM read ports (Pool TensorCopy from PSUM fails BIR verification).
COPYMAP = os.environ.get("AGC_COPYMAP", "v" * 13 + "a" * 3)

_cached = {}


def _flat_start_barrier(self, *, sem_only=False):
    """Flat all-engine barrier: every engine incs one sem and waits for the
    full count — one cross-engine hop instead of the stock sequential chain."""
    arrive = self.alloc_semaphore("flat_barrier_arrive")
    n = len(self.engines)
    for eng in self.engines.values():
        eng.sem_inc(arrive, 1)
    for eng in self.engines.values():
        eng.wait_ge(arrive, n)
    if not hasattr(self, "_flat_barrier_sems"):
        self._flat_barrier_sems = []
    self._flat_barrier_sems.append(arrive)


_stock_drain_and_barrier = tile.TileContext._drain_and_barrier


def _tail_drain_and_barrier(self, tick_clock, wait_clock):
    """Stock tail (its barrier instructions order the in-flight DMA completion
    sems ahead of the clears) + clear the flat-start-barrier sem so
    re-execution starts from zero."""
    _stock_drain_and_barrier(self, tick_clock, wait_clock)
    nc = self.nc
    fs = getattr(nc, "_flat_barrier_sems", [])
    if fs:
        nc.clear_and_free_semaphores(fs)
        nc._flat_barrier_sems = []


def build_nc():
    tile.TileContext._drain_and_barrier = _tail_drain_and_barrier
    f32 = mybir.dt.float32
    bf16 = mybir.dt.bfloat16
    orig_barrier = bass.Bass.all_engine_barrier
    if FLATBAR:
        bass.Bass.all_engine_barrier = _flat_start_barrier
    try:
        nc = bacc.Bacc("TRN2", target_bir_lowering=False, debug=False)
    finally:
        bass.Bass.all_engine_barrier = orig_barrier

    # V pre-scaled by sqrt(pi)/2, pre-cast to bf16 AND pre-tiled partition-
    # major on the host: Vp[p, s*D+d] = V[row 128s+p, d] — every partition is
    # one contiguous run per DMA slice (big descriptors -> full DMA bandwidth)
    vp_d = nc.dram_tensor("Vp", [128, NSLAB * D], bf16, kind="ExternalInput").ap()
    # cst = [iota(256) | b0' r' pairs] per partition
    cst_d = nc.dram_tensor("cst", [128, CW], f32, kind="ExternalInput").ap()
    out_d = nc.dram_tensor("out", [HALF, D], bf16, kind="ExternalOutput").ap()

    with tile.TileContext(nc) as tc:
        with (
            tc.tile_pool(name="const", bufs=1) as constp,
            tc.tile_pool(name="big", bufs=1) as bigp,
            tc.tile_pool(name="ps", bufs=8, space=bass.MemorySpace.PSUM) as pspool,
            tc.tile_pool(name="obuf", bufs=8) as opool,
        ):
            cst_t = constp.tile([128, CW], f32, name="cst_t")
            b0r = lambda s: (cst_t[:, 2 * s : 2 * s + 1],
                             cst_t[:, 2 * s + 1 : 2 * s + 2])

            vbuf = bigp.tile([128, NSLAB * D], bf16, name="vbuf")
            wbuf = bigp.tile([128, NSLAB * WWIN], bf16, name="wbuf")

            # cst (tiny b0'/r' table) as GpSimd's very first instruction:
            # gpsimd is ready ~1us before sync (sync runs a slow start-up
            # DRAIN), and being first keeps the W chain's one dependency off
            # the descriptor-generation queue behind V's 2.2MB
            nc.gpsimd.dma_start(cst_t[:], cst_d[:])

            # force the erf_derivative ACT table load NOW (it is inserted
            # right before the first activation in ACT program order; with a
            # no-dependency dummy first it runs at engine start instead of
            # after the cst semaphore wait)
            dummy = constp.tile([128, 1], f32, name="dummy")
            nc.scalar.activation(dummy[:], dummy[:], AF.Derivative_Erf)

            # iota source row for the W activations, generated on-chip
            # (fp32 is exact for 0..255)
            iota_t = constp.tile([128, WWIN], f32, name="iota_t")
            nc.gpsimd.iota(iota_t[:], [[1, WWIN]], base=0, channel_multiplier=0,
                           allow_small_or_imprecise_dtypes=True)

            # PE warm-up: scratch matmuls on zeros so the clock gate has
            # ramped when the real matmul stream begins.  Memsets on DVE —
            # it is otherwise idle until its first PSUM copy.
            wscr = bigp.tile([128, 128], bf16, name="wscr")
            nc.vector.memset(wscr[:], 0.0)
            wscr2 = bigp.tile([128, D], bf16, name="wscr2")
            nc.vector.memset(wscr2[:], 0.0)
            wps = pspool.tile([128, D], f32, tag="ps", name="wps")
            for _ in range(WARMUP):
                nc.tensor.matmul(wps[:, 0:D], wscr[:], wscr2[:],
                                 start=True, stop=True)

            # V loads all on the sync ring behind cst, slab order =
            # consumption order.  The out pairs ride the same queue behind
            # V, so V keeps priority.
            for lo, hi in ((0, 4), (4, 8), (8, 12), (12, 17)):
                nc.sync.dma_start(vbuf[:, lo * D : hi * D], vp_d[:, lo * D : hi * D])

            # W slab s in one ACT pass: DErf(r'*u + b0') = (2/sqrt(pi)) *
            # exp(-((u - 64 - p - mu)/sigma)^2 / 2)
            def emit_w(s):
                lo, w = _slab_win(s)
                b0, r = b0r(s)
                nc.scalar.activation(
                    wbuf[:, s * WWIN + lo : s * WWIN + lo + w],
                    iota_t[:, lo : lo + w],
                    AF.Derivative_Erf, bias=b0, scale=r)

            out3 = out_d.rearrange("(P h p) d -> P p h d", h=2, p=128)

            def emit_jtile(t, ps):
                out = ps[:]
                for k in range(2):
                    ls = t + k
                    nc.tensor.matmul(
                        out,
                        wbuf[:, ls * WWIN + (1 - k) * 128 : ls * WWIN + (2 - k) * 128],
                        vbuf[:, ls * D : (ls + 1) * D],
                        start=(k == 0),
                        stop=(k == 1),
                    )

            def emit_copy(t, ps, ob):
                dst = ob[:, (t % 2) * D : (t % 2 + 1) * D]
                if COPYMAP[t] == "a":
                    nc.scalar.activation(dst, ps[:], AF.Copy)
                else:
                    nc.vector.tensor_copy(dst, ps[:])

            def emit_out_dma(t, ob):
                if t == 14:
                    nc.sync.dma_start(out3[7, :, 0, :], ob[:, 0:D])
                elif t == 15:
                    nc.scalar.dma_start(out3[7, :, 1, :], ob[:, D : 2 * D])
                elif t % 2 == 1:
                    nc.sync.dma_start(
                        out3[t // 2],
                        ob[:].rearrange("p (h d) -> p h d", h=2))

            # pipeline: per-slab W -> j-tiles as they unlock -> copy -> DMA.
            # Out pairs ride the sync ring behind V (V keeps priority).
            # ACT-owned copies (and their DMAs) are deferred until after the
            # last W slab so the W stream never stalls PE.
            emit_w(0)
            ps = ob = None
            deferred = []
            for s in range(1, NSLAB):
                emit_w(s)
                t = s - 1
                ps = pspool.tile([128, D], f32, tag="ps", name="ps")
                if t % 2 == 0:
                    ob = opool.tile([128, 2 * D], bf16, name="ob")
                emit_jtile(t, ps)
                if COPYMAP[t] == "a":
                    deferred.append((t, ps, ob))
                else:
                    emit_copy(t, ps, ob)
                    emit_out_dma(t, ob)
            for t, ps, ob in deferred:
                emit_copy(t, ps, ob)
                emit_out_dma(t, ob)

    nc.compile()
    return nc


def _get_nc():
    if "nc" not in _cached:
        _cached["nc"] = build_nc()
    return _cached["nc"]


def make_in_maps(V, sigma, mu):
    """Host-side sharding: per-core padded bf16 V rows + scale table."""
    V = np.asarray(V, dtype=np.float32)
    sigma = np.asarray(sigma, dtype=np.float32).reshape(B, N)
    mu = np.asarray(mu, dtype=np.float32).reshape(B, N)
    pidx = (np.arange(VROWS) % 128).astype(np.float32)
    in_maps = []
    for c in range(NCORES):
        b, h = divmod(c, 2)
        jb = h * HALF
        lo, hi = jb - 64, jb + HALF + 64
        slo, shi = max(lo, 0), min(hi, N)
        vp = np.zeros((VROWS, D), ml_dtypes.bfloat16)
        sig = np.ones(VROWS, np.float32)
        muv = np.zeros(VROWS, np.float32)
        vp[slo - lo : shi - lo] = (V[b, slo:shi] * WSCALE).astype(ml_dtypes.bfloat16)
        sig[slo - lo : shi - lo] = sigma[b, slo:shi]
        muv[slo - lo : shi - lo] = mu[b, slo:shi]
        r = (np.float32(1.0) / (sig * np.float32(SQRT2))).astype(np.float32)
        b0 = ((np.float32(-64.0) - pidx - muv) * r).astype(np.float32)
        cst = np.zeros((128, CW), np.float32)
        cst[:, 0 : 2 * NSLAB : 2] = b0.reshape(NSLAB, 128).T
        cst[:, 1 : 2 * NSLAB : 2] = r.reshape(NSLAB, 128).T
        vp2 = np.ascontiguousarray(
            vp.reshape(NSLAB, 128, D).transpose(1, 0, 2).reshape(128, NSLAB * D))
        in_maps.append({"Vp": vp2, "cst": cst})
    return in_maps


def gather(results):
    out = np.empty((B, N, D), np.float32)
    for c in range(NCORES):
        b, h = divmod(c, 2)
        out[b, h * HALF : (h + 1) * HALF] = np.asarray(
            results[c]["out"]).astype(np.float32)
    return out


def kernel(V, sigma, mu):
    nc = _get_nc()
    in_maps = make_in_maps(V, sigma, mu)
    res = run_bass_kernel_spmd(nc, in_maps, core_ids=list(range(NCORES)))
    return gather(res.results)
